# revision 17
# baseline (speedup 1.0000x reference)
"""Trainium2 Bass kernel: 2D Haar DWT (single level) on x[8, 256, 256, 64] f32.

Math: with this problem's symmetric-pad + stride-2 slicing, the padding never
contributes; each output element is a +/- combination of one 2x2 spatial block:
    p = x[2i, 2j], q = x[2i, 2j+1], r = x[2i+1, 2j], s = x[2i+1, 2j+1]
    ll = 0.5(p+q+r+s)   -> out[0:128, 0:128]
    lh = 0.5(p+q-r-s)   -> out[128:256, 0:128]
    hl = 0.5(p-q+r-s)   -> out[0:128, 128:256]
    hh = 0.5(p-q-r+s)   -> out[128:256, 128:256]
(per channel; channels are the contiguous innermost dim)

Sharding: pure data-parallel, one batch example per NeuronCore (8 cores).

Per-core kernel layout: partition dim = i (the 128 H-pairs). Each iteration
handles a chunk of WC output columns:
  - one DMA loads rows 2i and 2i+1 of the input W-chunk into X[128, 2*WC*128]
  - DVE scales the odd-row half by 0.5 in place (so the butterfly can fold the
    remaining 0.5 into scalar_tensor_tensor's scalar operand); DVE is the only
    engine touching X, which keeps per-instruction sync-wait counts within the
    ISA encoding limits
  - DVE: stage 1 = two tensor ops (W-direction sums/diffs for both row
    parities at once), stage 2 = four scalar_tensor_tensor ops writing the
    quadrant results into an OUT tile
  - one DMA (ACT ring) stores all four quadrant chunks
"""

import numpy as np

import concourse.bacc as bacc
import concourse.mybir as mybir
from concourse import bass_utils
from concourse.tile import TileContext

B, H, W, C = 8, 256, 256, 64
ROW = W * C          # 16384 f32 per input row
# output j-columns per iteration: small first/last chunks shrink the
# head (first load before DVE can start) and tail (last store) ramps
CHUNKS = [8, 12, 28, 28, 28, 16, 8]
WCMAX = max(CHUNKS)

F32 = mybir.dt.float32
ADD = mybir.AluOpType.add
SUB = mybir.AluOpType.subtract
MUL = mybir.AluOpType.mult


def _dwt_tile_kernel(tc, out, x):
    nc = tc.nc
    # x, out: DRAM APs of shape (256, 16384)
    xr = x.rearrange("(i hp) w -> i hp w", hp=2)            # (128, 2, 16384)
    outr = out.rearrange("(qh i) (qw e) -> i qw qh e", qh=2, qw=2)  # (128,2,2,8192)

    xwmax = 2 * WCMAX * C

    with (
        tc.tile_pool(name="px", bufs=3) as px,
        tc.tile_pool(name="pm", bufs=1) as pm,
        tc.tile_pool(name="po", bufs=3) as po,
    ):
        j0 = 0
        for it, WC in enumerate(CHUNKS):
            xw = 2 * WC * C   # input elems per row per chunk
            ow = WC * C       # output elems per quadrant per chunk
            xt = px.tile([128, 2 * xw], F32, name=f"xt{it}", tag="xt",
                         padded_shape=[128, 2 * xwmax])
            md = pm.tile([128, 2 * xw], F32, name=f"md{it}", tag="md",
                         padded_shape=[128, 2 * xwmax])
            ot = po.tile([128, 2 * xw], F32, name=f"ot{it}", tag="ot",
                         padded_shape=[128, 2 * xwmax])

            # load rows 2i (-> xt[:, :xw]) and 2i+1 (-> xt[:, xw:])
            xo = xt[:, xw:]
            if it == 0:
                # head chunk: odd rows first + DVE scale, so the scale hides
                # under the even-row load and the first TT starts sooner
                nc.sync.dma_start(out=xo, in_=xr[:, 1, 2 * j0 * C : 2 * j0 * C + xw])
                nc.sync.dma_start(
                    out=xt[:, :xw], in_=xr[:, 0, 2 * j0 * C : 2 * j0 * C + xw]
                )
                nc.vector.tensor_scalar_mul(xo, xo, 0.5)
            else:
                nc.sync.dma_start(
                    out=xt.rearrange("p (hp e) -> p hp e", hp=2),
                    in_=xr[:, :, 2 * j0 * C : 2 * j0 * C + xw],
                )
                # ACT: scale odd rows by 0.5 in place (bacc splits multi-waits)
                nc.scalar.mul(xo, xo, 0.5)

            # stage 1 (DVE): W-direction butterfly for both row parities.
            # md layout: [a | b' | d | e'] (2048 each)
            x5 = xt.rearrange("p (hp jl dj c) -> p hp jl dj c", hp=2, jl=WC, dj=2, c=C)
            ev, od = x5[:, :, :, 0, :], x5[:, :, :, 1, :]
            ab4 = md[:, :xw].rearrange("p (hp jl c) -> p hp jl c", hp=2, jl=WC, c=C)
            de4 = md[:, xw:].rearrange("p (hp jl c) -> p hp jl c", hp=2, jl=WC, c=C)
            nc.vector.tensor_add(out=ab4, in0=ev, in1=od)   # [a | b'] = [p+q | .5r+.5s]
            nc.vector.tensor_sub(out=de4, in0=ev, in1=od)   # [d | e'] = [p-q | .5r-.5s]

            # stage 2 (DVE): two ops, each covering two quadrants via the
            # grouped view g in {(a,b')->ll/lh, (d,e')->hl/hh}:
            #   in0 = [a | d]  (stride 2*ow), in1 = [b' | e'], 0.5 fused on in0
            # OUT layout [ll | lh | hl | hh] lines up with g stride 2*ow.
            in0 = md.rearrange("p (g two e) -> p g two e", g=2, two=2)[:, :, 0, :]
            in1 = md.rearrange("p (g two e) -> p g two e", g=2, two=2)[:, :, 1, :]
            og = ot.rearrange("p (g two e) -> p g two e", g=2, two=2)
            nc.vector.scalar_tensor_tensor(
                out=og[:, :, 0, :], in0=in0, scalar=0.5, in1=in1, op0=MUL, op1=ADD)
            nc.vector.scalar_tensor_tensor(
                out=og[:, :, 1, :], in0=in0, scalar=0.5, in1=in1, op0=MUL, op1=SUB)

            # store all 4 quadrant chunks in one DMA (ACT ring, so a store's
            # wait never blocks the next load's issue on SP)
            nc.scalar.dma_start(
                out=outr[:, :, :, j0 * C : j0 * C + ow],
                in_=ot.rearrange("p (qw qh e) -> p qw qh e", qw=2, qh=2),
            )
            j0 += WC


_NC_CACHE = None


def _get_nc():
    global _NC_CACHE
    if _NC_CACHE is None:
        nc = bacc.Bacc("TRN2", target_bir_lowering=False, debug=False)
        x = nc.dram_tensor("x", [H, ROW], F32, kind="ExternalInput").ap()
        out = nc.dram_tensor("out", [H, ROW], F32, kind="ExternalOutput").ap()
        with TileContext(nc) as tc:
            _dwt_tile_kernel(tc, out, x)
        nc.compile()  # bacc passes: splits multi-waits into event semaphores etc.
        _NC_CACHE = nc
    return _NC_CACHE


def kernel(x: np.ndarray) -> np.ndarray:
    assert x.shape == (B, H, W, C), x.shape
    nc = _get_nc()
    in_maps = [
        {"x": np.ascontiguousarray(x[b], dtype=np.float32).reshape(H, ROW)}
        for b in range(B)
    ]
    res = bass_utils.run_bass_kernel_spmd(nc, in_maps, core_ids=list(range(B)))
    return np.stack(
        [r["out"].reshape(H, W, C) for r in res.results], axis=0
    ).astype(x.dtype, copy=False)


# revision 18
# speedup vs baseline: 1.1161x; 1.1161x over previous
"""Trainium2 Bass kernel: 2D Haar DWT (single level) on x[8, 256, 256, 64] f32.

Math: with this problem's symmetric-pad + stride-2 slicing, the padding never
contributes; each output element is a +/- combination of one 2x2 spatial block:
    p = x[2i, 2j], q = x[2i, 2j+1], r = x[2i+1, 2j], s = x[2i+1, 2j+1]
    ll = 0.5(p+q+r+s)   -> out[0:128, 0:128]
    lh = 0.5(p+q-r-s)   -> out[128:256, 0:128]
    hl = 0.5(p-q+r-s)   -> out[0:128, 128:256]
    hh = 0.5(p-q-r+s)   -> out[128:256, 128:256]
(per channel; channels are the contiguous innermost dim)

Sharding: pure data-parallel, one batch example per NeuronCore (8 cores).

Per-core kernel layout: partition dim = i (the 128 H-pairs). Each iteration
handles a chunk of WC output columns:
  - one DMA loads rows 2i and 2i+1 of the input W-chunk into X[128, 2*WC*128]
  - DVE scales the odd-row half by 0.5 in place (so the butterfly can fold the
    remaining 0.5 into scalar_tensor_tensor's scalar operand); DVE is the only
    engine touching X, which keeps per-instruction sync-wait counts within the
    ISA encoding limits
  - DVE: stage 1 = two tensor ops (W-direction sums/diffs for both row
    parities at once), stage 2 = four scalar_tensor_tensor ops writing the
    quadrant results into an OUT tile
  - one DMA (ACT ring) stores all four quadrant chunks
"""

import numpy as np

import concourse.bacc as bacc
import concourse.mybir as mybir
from concourse import bass_utils
from concourse.tile import TileContext

B, H, W, C = 8, 256, 256, 64
ROW = W * C          # 16384 f32 per input row
# output j-columns per iteration: small first/last chunks shrink the
# head (first load before DVE can start) and tail (last store) ramps
CHUNKS = [8, 16, 28, 28, 28, 12, 8]
WCMAX = max(CHUNKS)

F32 = mybir.dt.float32
ADD = mybir.AluOpType.add
SUB = mybir.AluOpType.subtract
MUL = mybir.AluOpType.mult


def _dwt_tile_kernel(tc, out, x):
    nc = tc.nc
    # x, out: DRAM APs of shape (256, 16384)
    xr = x.rearrange("(i hp) w -> i hp w", hp=2)            # (128, 2, 16384)
    outr = out.rearrange("(qh i) (qw e) -> i qw qh e", qh=2, qw=2)  # (128,2,2,8192)

    xwmax = 2 * WCMAX * C

    with (
        tc.tile_pool(name="px", bufs=3) as px,
        tc.tile_pool(name="pm", bufs=1) as pm,
        tc.tile_pool(name="po", bufs=3) as po,
    ):
        j0 = 0
        for it, WC in enumerate(CHUNKS):
            xw = 2 * WC * C   # input elems per row per chunk
            ow = WC * C       # output elems per quadrant per chunk
            xt = px.tile([128, 2 * xw], F32, name=f"xt{it}", tag="xt",
                         padded_shape=[128, 2 * xwmax])
            md = pm.tile([128, 2 * xw], F32, name=f"md{it}", tag="md",
                         padded_shape=[128, 2 * xwmax])
            ot = po.tile([128, 2 * xw], F32, name=f"ot{it}", tag="ot",
                         padded_shape=[128, 2 * xwmax])

            # load rows 2i (-> xt[:, :xw]) and 2i+1 (-> xt[:, xw:])
            xo = xt[:, xw:]
            if it == 0:
                # head chunk: odd rows first + DVE scale, so the scale hides
                # under the even-row load and the first TT starts sooner
                nc.sync.dma_start(out=xo, in_=xr[:, 1, 2 * j0 * C : 2 * j0 * C + xw])
                nc.sync.dma_start(
                    out=xt[:, :xw], in_=xr[:, 0, 2 * j0 * C : 2 * j0 * C + xw]
                )
                nc.vector.tensor_scalar_mul(xo, xo, 0.5)
            else:
                nc.sync.dma_start(
                    out=xt.rearrange("p (hp e) -> p hp e", hp=2),
                    in_=xr[:, :, 2 * j0 * C : 2 * j0 * C + xw],
                )
                # ACT: scale odd rows by 0.5 in place (bacc splits multi-waits)
                nc.scalar.mul(xo, xo, 0.5)

            # stage 1 (DVE): W-direction butterfly for both row parities.
            # md layout: [a | b' | d | e'] (2048 each)
            x5 = xt.rearrange("p (hp jl dj c) -> p hp jl dj c", hp=2, jl=WC, dj=2, c=C)
            ev, od = x5[:, :, :, 0, :], x5[:, :, :, 1, :]
            ab4 = md[:, :xw].rearrange("p (hp jl c) -> p hp jl c", hp=2, jl=WC, c=C)
            de4 = md[:, xw:].rearrange("p (hp jl c) -> p hp jl c", hp=2, jl=WC, c=C)
            nc.vector.tensor_add(out=ab4, in0=ev, in1=od)   # [a | b'] = [p+q | .5r+.5s]
            nc.vector.tensor_sub(out=de4, in0=ev, in1=od)   # [d | e'] = [p-q | .5r-.5s]

            # stage 2 (DVE): two ops, each covering two quadrants via the
            # grouped view g in {(a,b')->ll/lh, (d,e')->hl/hh}:
            #   in0 = [a | d]  (stride 2*ow), in1 = [b' | e'], 0.5 fused on in0
            # OUT layout [ll | lh | hl | hh] lines up with g stride 2*ow.
            in0 = md.rearrange("p (g two e) -> p g two e", g=2, two=2)[:, :, 0, :]
            in1 = md.rearrange("p (g two e) -> p g two e", g=2, two=2)[:, :, 1, :]
            og = ot.rearrange("p (g two e) -> p g two e", g=2, two=2)
            nc.vector.scalar_tensor_tensor(
                out=og[:, :, 0, :], in0=in0, scalar=0.5, in1=in1, op0=MUL, op1=ADD)
            nc.vector.scalar_tensor_tensor(
                out=og[:, :, 1, :], in0=in0, scalar=0.5, in1=in1, op0=MUL, op1=SUB)

            # store all 4 quadrant chunks in one DMA (ACT ring, so a store's
            # wait never blocks the next load's issue on SP)
            nc.scalar.dma_start(
                out=outr[:, :, :, j0 * C : j0 * C + ow],
                in_=ot.rearrange("p (qw qh e) -> p qw qh e", qw=2, qh=2),
            )
            j0 += WC


_NC_CACHE = None


def _get_nc():
    global _NC_CACHE
    if _NC_CACHE is None:
        nc = bacc.Bacc("TRN2", target_bir_lowering=False, debug=False)
        x = nc.dram_tensor("x", [H, ROW], F32, kind="ExternalInput").ap()
        out = nc.dram_tensor("out", [H, ROW], F32, kind="ExternalOutput").ap()
        with TileContext(nc) as tc:
            _dwt_tile_kernel(tc, out, x)
        nc.compile()  # bacc passes: splits multi-waits into event semaphores etc.
        _NC_CACHE = nc
    return _NC_CACHE


def kernel(x: np.ndarray) -> np.ndarray:
    assert x.shape == (B, H, W, C), x.shape
    nc = _get_nc()
    in_maps = [
        {"x": np.ascontiguousarray(x[b], dtype=np.float32).reshape(H, ROW)}
        for b in range(B)
    ]
    res = bass_utils.run_bass_kernel_spmd(nc, in_maps, core_ids=list(range(B)))
    return np.stack(
        [r["out"].reshape(H, W, C) for r in res.results], axis=0
    ).astype(x.dtype, copy=False)
